# revision 1
# baseline (speedup 1.0000x reference)
"""nn_DenseGrid trilinear embedding lookup on 8 Trainium2 cores.

Strategy (data-parallel over points, codebook replicated per core):
  - 2,097,152 points sharded 8 ways (262,144 per core); full output gathered
    on host by concatenation.
  - Per core, points are processed in super-chunks of 128*F (partition p,
    slot f). For each point: fold transform+scale into q = A@p + b, floor
    (magic-number round + fixup, no reliance on HW cast rounding mode),
    fractional weights, base row index = x + 128y + 16384z.
  - A y-pair table P2[j] = [cb[j], cb[j+128]] (2x codebook) is built once
    per core with large contiguous DMAs + on-chip DVE interleave. One 288B
    gather descriptor starting at entry j then covers all 4 xy-corners of
    cell j, so each point needs only 2 descriptors (z0, z1).
  - Gather: indirect DMA, one descriptor per partition per instruction (the
    only mode trn2 walrus supports; ~1.45us per 128 descriptors, so
    instruction count dominates the runtime).
  - Interpolation: G *= W8 (8 corner weights broadcast over 18 features),
    then in-place tree reduction 144 -> 72 -> 36 -> 18 per point; strided
    store back to DRAM.
"""

import numpy as np

RES = 128
FEAT = 18
V = RES**3
MAGIC = float(2**23)
P = 128
N_CORES = 8
F = 64                      # point slots per partition per super-chunk

_cache = {}


def _build(n_points, A, b):
    import os
    os.environ.setdefault("NEURON_SCRATCHPAD_PAGE_SIZE", "320")
    import concourse.bass as bass
    import concourse.bacc as bacc
    import concourse.mybir as mybir
    import concourse.tile as tile

    f32 = mybir.dt.float32
    i32 = mybir.dt.int32
    Copy = mybir.ActivationFunctionType.Copy
    Op = mybir.AluOpType

    chunk = P * F
    n_chunks = n_points // chunk
    assert n_chunks * chunk == n_points

    nc = bacc.Bacc(None, target_bir_lowering=False, debug=False)
    pts = nc.declare_dram_parameter("pts", [n_points, 3], f32, isOutput=False)
    cb = nc.declare_dram_parameter("codebook", [V, FEAT], f32, isOutput=False)
    out = nc.declare_dram_parameter("out", [n_points, FEAT], f32, isOutput=True)

    # y-pair table: P2[j] = [cb[j], cb[j+128]] (36 floats). One 72-float
    # descriptor starting at entry j covers entries j, j+1 = the 4 xy-corners
    # (x0y0, x0y1, x1y0, x1y1) of cell base row j.
    p2 = nc.dram_tensor("p2tab", [V, 2 * FEAT], f32)
    with tile.TileContext(nc) as tc:
        with (
            tc.tile_pool(name="build", bufs=2) as bpool,
            tc.tile_pool(name="g", bufs=2) as gpool,
            tc.tile_pool(name="small", bufs=2) as spool,
        ):
            # Interleave on-chip: load rows [r0, r0+R) and [r0+128, r0+128+R)
            # into two tiles (pairs line up per partition), DVE-interleave into
            # [row, pair, 18], store contiguous. All DMAs are big & contiguous.
            ROWS = 8192
            RPP = ROWS // P
            n_bchunks = V // ROWS
            for ci in range(n_bchunks):
                r0 = ci * ROWS
                At = bpool.tile([P, RPP * FEAT], f32, tag="BA")
                Bt = bpool.tile([P, RPP * FEAT], f32, tag="BB")
                nc.scalar.dma_start(
                    out=At[:],
                    in_=cb[r0 : r0 + ROWS, :].rearrange("(p r) e -> p (r e)", p=P))
                if ci < n_bchunks - 1:
                    nc.scalar.dma_start(
                        out=Bt[:],
                        in_=cb[r0 + P : r0 + P + ROWS, :].rearrange("(p r) e -> p (r e)", p=P))
                else:
                    # last 128 pair rows are out of range (y=127 entries,
                    # never indexed) -> zero-fill
                    nc.vector.memset(Bt[:], 0.0)
                    nc.scalar.dma_start(
                        out=Bt[:126, :],
                        in_=cb[r0 + P : V, :].rearrange("(p r) e -> p (r e)", p=126))
                Ot = bpool.tile([P, RPP, 2, FEAT], f32, tag="BO")
                nc.vector.tensor_copy(out=Ot[:, :, 0, :],
                                      in_=At[:].rearrange("p (r e) -> p r e", e=FEAT))
                nc.vector.tensor_copy(out=Ot[:, :, 1, :],
                                      in_=Bt[:].rearrange("p (r e) -> p r e", e=FEAT))
                nc.sync.dma_start(
                    out=p2[r0 : r0 + ROWS, :].rearrange("(p r) e -> p (r e)", p=P),
                    in_=Ot[:].rearrange("p r t e -> p (r t e)"))
            for c in range(n_chunks):
                c0 = c * chunk
                PT = spool.tile([P, 3 * F], f32, tag="PT")
                nc.sync.dma_start(
                    out=PT[:],
                    in_=pts[c0 : c0 + chunk, :].rearrange("(p f) c -> p (f c)", p=P),
                )
                PT3 = PT[:].rearrange("p (f c) -> p f c", c=3)

                Q = spool.tile([P, 3, F], f32, tag="Q")
                FL = spool.tile([P, 3, F], f32, tag="FL")
                W = spool.tile([P, 3, F], f32, tag="W")
                U = spool.tile([P, 3, F], f32, tag="U")
                T = spool.tile([P, 3, F], f32, tag="T")
                # q_k = A[k,0]x + A[k,1]y + A[k,2]z + b_k
                for k in range(3):
                    nc.scalar.activation(Q[:, k, :], PT3[:, :, 0], Copy,
                                         bias=float(b[k]), scale=float(A[k][0]))
                    nc.scalar.activation(T[:, k, :], PT3[:, :, 1], Copy,
                                         bias=0.0, scale=float(A[k][1]))
                    nc.vector.tensor_tensor(out=Q[:, k, :], in0=Q[:, k, :], in1=T[:, k, :], op=Op.add)
                    nc.scalar.activation(T[:, k, :], PT3[:, :, 2], Copy,
                                         bias=0.0, scale=float(A[k][2]))
                    nc.vector.tensor_tensor(out=Q[:, k, :], in0=Q[:, k, :], in1=T[:, k, :], op=Op.add)
                # floor(q): round-to-nearest via magic constant, then fix up
                nc.scalar.activation(T[:], Q[:], Copy, bias=MAGIC)
                nc.scalar.activation(FL[:], T[:], Copy, bias=-MAGIC)
                nc.vector.tensor_tensor(out=T[:], in0=FL[:], in1=Q[:], op=Op.is_gt)
                nc.vector.tensor_tensor(out=FL[:], in0=FL[:], in1=T[:], op=Op.subtract)
                # frac weights (from unclipped floor), then clip floor to [0,126]
                nc.vector.tensor_tensor(out=W[:], in0=Q[:], in1=FL[:], op=Op.subtract)
                nc.vector.tensor_scalar(out=FL[:], in0=FL[:], scalar1=0.0, scalar2=float(RES - 2),
                                        op0=Op.max, op1=Op.min)
                nc.scalar.activation(U[:], W[:], Copy, bias=1.0, scale=-1.0)

                # xy corner weights (dx major, dy minor), then scale by z
                W4 = spool.tile([P, 4, F], f32, tag="W4")
                nc.vector.tensor_tensor(out=W4[:, 0, :], in0=U[:, 0, :], in1=U[:, 1, :], op=Op.mult)
                nc.vector.tensor_tensor(out=W4[:, 1, :], in0=U[:, 0, :], in1=W[:, 1, :], op=Op.mult)
                nc.vector.tensor_tensor(out=W4[:, 2, :], in0=W[:, 0, :], in1=U[:, 1, :], op=Op.mult)
                nc.vector.tensor_tensor(out=W4[:, 3, :], in0=W[:, 0, :], in1=W[:, 1, :], op=Op.mult)
                W8 = spool.tile([P, F, 8], f32, tag="W8")
                for k in range(4):
                    nc.vector.tensor_tensor(out=W8[:, :, k], in0=W4[:, k, :], in1=U[:, 2, :], op=Op.mult)
                    nc.vector.tensor_tensor(out=W8[:, :, 4 + k], in0=W4[:, k, :], in1=W[:, 2, :], op=Op.mult)

                # base row index = fx + 128 fy + 16384 fz  (exact in f32)
                B = spool.tile([P, F], f32, tag="B")
                T2 = spool.tile([P, 2, F], f32, tag="T2")
                nc.scalar.activation(T2[:, 0, :], FL[:, 1, :], Copy, scale=float(RES))
                nc.scalar.activation(T2[:, 1, :], FL[:, 2, :], Copy, scale=float(RES * RES))
                nc.vector.tensor_tensor(out=B[:], in0=FL[:, 0, :], in1=T2[:, 0, :], op=Op.add)
                nc.vector.tensor_tensor(out=B[:], in0=B[:], in1=T2[:, 1, :], op=Op.add)
                IDX = spool.tile([P, F, 2], i32, tag="IDX")
                nc.vector.tensor_copy(out=IDX[:, :, 0], in_=B[:])
                nc.vector.tensor_scalar(out=IDX[:, :, 1], in0=B[:], scalar1=float(RES * RES),
                                        scalar2=None, op0=Op.add)

                # gather: per point-slot f, per z-plane: 72 floats = 4 xy corners
                G = gpool.tile([P, F, 2, 72], f32, tag="G")
                for g in range(F):
                    for zz in range(2):
                        nc.gpsimd.indirect_dma_start(
                            out=G[:, g, zz, :],
                            out_offset=None,
                            in_=p2[:],
                            in_offset=bass.IndirectOffsetOnAxis(ap=IDX[:, g, zz : zz + 1], axis=0),
                        )

                # weighted multiply + in-place tree reduction
                Gv = G[:].rearrange("p f z e -> p (f z e)").rearrange(
                    "p (f d j) -> p f d j", d=8, j=FEAT)
                W8b = W8[:].unsqueeze(-1).broadcast_to([P, F, 8, FEAT])
                nc.vector.tensor_tensor(out=Gv, in0=Gv, in1=W8b, op=Op.mult)
                Gf = G[:].rearrange("p f z e -> p (f z e)")
                for width in (72, 36, 18):
                    a = Gf.rearrange("p (f e) -> p f e", e=144)[:, :, 0:width]
                    bb = Gf.rearrange("p (f e) -> p f e", e=144)[:, :, width : 2 * width]
                    nc.vector.tensor_tensor(out=a, in0=a, in1=bb, op=Op.add)

                res = Gf.rearrange("p (f e) -> p f e", e=144)[:, :, 0:FEAT]
                nc.sync.dma_start(
                    out=out[c0 : c0 + chunk, :].rearrange("(p f) c -> p (f c)", p=P),
                    in_=res,
                )
    nc.finalize()
    return nc


def kernel(pts, codebook, transform, _trace=False):
    from concourse.bass_utils import run_bass_kernel_spmd

    pts = np.asarray(pts, dtype=np.float32)
    codebook = np.ascontiguousarray(np.asarray(codebook, dtype=np.float32))
    transform = np.asarray(transform, dtype=np.float32)

    p_flat = np.ascontiguousarray(pts.reshape(-1, 3))
    n_total = p_flat.shape[0]
    n_per = n_total // N_CORES
    assert n_per * N_CORES == n_total

    # fold transform inverse + grid scale into affine q = A p + b (host side,
    # 4x4 input only)
    R_inv = np.linalg.inv(transform[:3, :3].astype(np.float64))
    A = (RES - 1) * R_inv
    b = -A @ transform[:3, 3].astype(np.float64)

    key = (n_per, A.tobytes(), b.tobytes())
    if key not in _cache:
        _cache[key] = _build(n_per, A, b)
    nc = _cache[key]

    in_maps = [
        {"pts": p_flat[i * n_per : (i + 1) * n_per], "codebook": codebook}
        for i in range(N_CORES)
    ]
    r = run_bass_kernel_spmd(nc, in_maps, list(range(N_CORES)), trace=_trace)
    kernel.last_exec_time_ns = r.exec_time_ns
    out = np.concatenate([r.results[i]["out"] for i in range(N_CORES)], axis=0)
    return out


kernel.last_exec_time_ns = None



# revision 2
# speedup vs baseline: 1.5453x; 1.5453x over previous
"""nn_DenseGrid trilinear embedding lookup on 8 Trainium2 cores.

Strategy (data-parallel over points, codebook replicated per core):
  - 2,097,152 points sharded 8 ways (262,144 per core); full output gathered
    on host by concatenation.
  - Indirect-DMA cost model: ~994 ns fixed + 0.34 ns/descriptor per
    instruction, max 128 random descriptors (one per partition). So the
    gather is instruction-count bound -> minimize descriptors per point.
  - A z+y-pair table p4[j] = [cb[j], cb[j+128], cb[j+16384], cb[j+16512]]
    in bf16 (144 B/row) is built on device. One 576 B descriptor starting
    at row j covers rows j, j+1 = all 8 corners of cell j -> ONE descriptor
    per point (2048 indirect instructions total per core).
  - Corner order in the gathered row pair: c = dx*4 + dz*2 + dy.
  - Interpolation: G(bf16) * W8(bf16) -> f32, in-place tree reduction
    144 -> 72 -> 36 -> 18, compact, store.
"""

import numpy as np

RES = 128
FEAT = 18
V = RES**3
MAGIC = float(2**23)
P = 128
N_CORES = 8
F = 64                      # point slots per partition per chunk

_cache = {}


def _build(n_points, A, b):
    import os
    os.environ.setdefault("NEURON_SCRATCHPAD_PAGE_SIZE", "320")
    import concourse.bass as bass
    import concourse.bacc as bacc
    import concourse.mybir as mybir
    import concourse.tile as tile

    f32 = mybir.dt.float32
    bf16 = mybir.dt.bfloat16
    i32 = mybir.dt.int32
    Copy = mybir.ActivationFunctionType.Copy
    Op = mybir.AluOpType

    chunk = P * F
    n_chunks = n_points // chunk
    assert n_chunks * chunk == n_points

    nc = bacc.Bacc(None, target_bir_lowering=False, debug=False)
    pts = nc.declare_dram_parameter("pts", [n_points, 3], f32, isOutput=False)
    cb = nc.declare_dram_parameter("codebook", [V, FEAT], f32, isOutput=False)
    out = nc.declare_dram_parameter("out", [n_points, FEAT], f32, isOutput=True)

    # p4[j] = [cb[j], cb[j+128], cb[j+16384], cb[j+16512]] in bf16.
    # Indexed j range: j <= 2080639 (z <= 126, +1 for the x-pair), so only
    # rows [0, 127*16384) need valid data; the +16384/+16512 shifts then
    # never read past V except in the final block's tail (z=127 rows,
    # never indexed) which is zero-filled.
    NROWS = 127 * RES * RES             # 2080768 rows built
    p4 = nc.dram_tensor("p4tab", [V, 4 * FEAT], bf16)
    SH = [0, RES, RES * RES, RES * RES + RES]   # row shifts per corner pair
    with tile.TileContext(nc) as tc:
        with (
            tc.tile_pool(name="build", bufs=2) as bpool,
            tc.tile_pool(name="g", bufs=2) as gpool,
            tc.tile_pool(name="small", bufs=2) as spool,
            tc.tile_pool(name="t", bufs=2) as tpool,
        ):
            ROWS = 8192
            RPP = ROWS // P
            n_bchunks = NROWS // ROWS
            assert n_bchunks * ROWS == NROWS
            for ci in range(n_bchunks):
                r0 = ci * ROWS
                Ot = bpool.tile([P, RPP, 4, FEAT], bf16, tag="BO")
                for k, sh in enumerate(SH):
                    At = bpool.tile([P, RPP * FEAT], f32, tag=f"BA{k}")
                    lo = r0 + sh
                    if lo + ROWS <= V:
                        nc.scalar.dma_start(
                            out=At[:],
                            in_=cb[lo : lo + ROWS, :].rearrange(
                                "(p r) e -> p (r e)", p=P))
                    else:
                        # tail of the last block: rows >= V are never
                        # indexed (z=127); zero-fill then partial load
                        nfull = (V - lo) // RPP
                        nc.vector.memset(At[:], 0.0)
                        nc.scalar.dma_start(
                            out=At[:nfull, :],
                            in_=cb[lo : lo + nfull * RPP, :].rearrange(
                                "(p r) e -> p (r e)", p=nfull))
                    nc.vector.tensor_copy(
                        out=Ot[:, :, k, :],
                        in_=At[:].rearrange("p (r e) -> p r e", e=FEAT))
                nc.sync.dma_start(
                    out=p4[r0 : r0 + ROWS, :].rearrange("(p r) e -> p (r e)", p=P),
                    in_=Ot[:].rearrange("p r k e -> p (r k e)"))

            for c in range(n_chunks):
                c0 = c * chunk
                PT = spool.tile([P, 3 * F], f32, tag="PT")
                nc.sync.dma_start(
                    out=PT[:],
                    in_=pts[c0 : c0 + chunk, :].rearrange("(p f) c -> p (f c)", p=P),
                )
                PT3 = PT[:].rearrange("p (f c) -> p f c", c=3)

                Q = spool.tile([P, 3, F], f32, tag="Q")
                FL = spool.tile([P, 3, F], f32, tag="FL")
                W = spool.tile([P, 3, F], f32, tag="W")
                U = spool.tile([P, 3, F], f32, tag="U")
                T = spool.tile([P, 3, F], f32, tag="T")
                # q_k = A[k,0]x + A[k,1]y + A[k,2]z + b_k
                for k in range(3):
                    nc.scalar.activation(Q[:, k, :], PT3[:, :, 0], Copy,
                                         bias=float(b[k]), scale=float(A[k][0]))
                    nc.scalar.activation(T[:, k, :], PT3[:, :, 1], Copy,
                                         bias=0.0, scale=float(A[k][1]))
                    nc.vector.tensor_tensor(out=Q[:, k, :], in0=Q[:, k, :], in1=T[:, k, :], op=Op.add)
                    nc.scalar.activation(T[:, k, :], PT3[:, :, 2], Copy,
                                         bias=0.0, scale=float(A[k][2]))
                    nc.vector.tensor_tensor(out=Q[:, k, :], in0=Q[:, k, :], in1=T[:, k, :], op=Op.add)
                # floor(q): round-to-nearest via magic constant, then fix up
                nc.scalar.activation(T[:], Q[:], Copy, bias=MAGIC)
                nc.scalar.activation(FL[:], T[:], Copy, bias=-MAGIC)
                nc.vector.tensor_tensor(out=T[:], in0=FL[:], in1=Q[:], op=Op.is_gt)
                nc.vector.tensor_tensor(out=FL[:], in0=FL[:], in1=T[:], op=Op.subtract)
                # frac weights (from unclipped floor), then clip floor to [0,126]
                nc.vector.tensor_tensor(out=W[:], in0=Q[:], in1=FL[:], op=Op.subtract)
                nc.vector.tensor_scalar(out=FL[:], in0=FL[:], scalar1=0.0, scalar2=float(RES - 2),
                                        op0=Op.max, op1=Op.min)
                nc.scalar.activation(U[:], W[:], Copy, bias=1.0, scale=-1.0)

                # corner weights, order c = dx*4 + dz*2 + dy
                WXZ = spool.tile([P, 4, F], f32, tag="WXZ")
                nc.vector.tensor_tensor(out=WXZ[:, 0, :], in0=U[:, 0, :], in1=U[:, 2, :], op=Op.mult)
                nc.vector.tensor_tensor(out=WXZ[:, 1, :], in0=U[:, 0, :], in1=W[:, 2, :], op=Op.mult)
                nc.vector.tensor_tensor(out=WXZ[:, 2, :], in0=W[:, 0, :], in1=U[:, 2, :], op=Op.mult)
                nc.vector.tensor_tensor(out=WXZ[:, 3, :], in0=W[:, 0, :], in1=W[:, 2, :], op=Op.mult)
                W8 = spool.tile([P, F, 4, 2], bf16, tag="W8")
                for dx in range(2):
                    for dz in range(2):
                        k = dx * 2 + dz
                        nc.vector.tensor_tensor(out=W8[:, :, k, 0], in0=WXZ[:, k, :], in1=U[:, 1, :], op=Op.mult)
                        nc.vector.tensor_tensor(out=W8[:, :, k, 1], in0=WXZ[:, k, :], in1=W[:, 1, :], op=Op.mult)

                # base row index = fx + 128 fy + 16384 fz  (exact in f32)
                B = spool.tile([P, F], f32, tag="B")
                T2 = spool.tile([P, 2, F], f32, tag="T2")
                nc.scalar.activation(T2[:, 0, :], FL[:, 1, :], Copy, scale=float(RES))
                nc.scalar.activation(T2[:, 1, :], FL[:, 2, :], Copy, scale=float(RES * RES))
                nc.vector.tensor_tensor(out=B[:], in0=FL[:, 0, :], in1=T2[:, 0, :], op=Op.add)
                nc.vector.tensor_tensor(out=B[:], in0=B[:], in1=T2[:, 1, :], op=Op.add)
                IDX = spool.tile([P, F], i32, tag="IDX")
                nc.vector.tensor_copy(out=IDX[:], in_=B[:])

                # gather: one 576 B descriptor per point covers all 8 corners
                G = gpool.tile([P, F, 2, 4, FEAT], bf16, tag="G")
                for g in range(F):
                    nc.gpsimd.indirect_dma_start(
                        out=G[:, g, :, :, :].rearrange("p x k e -> p (x k e)"),
                        out_offset=None,
                        in_=p4[:],
                        in_offset=bass.IndirectOffsetOnAxis(ap=IDX[:, g : g + 1], axis=0),
                    )

                # weighted multiply into f32, then in-place tree reduction
                # G layout per point: [dx, dzdy(4), FEAT]; weight c = dx*4+dz*2+dy
                TT = tpool.tile([P, F, 8, FEAT], f32, tag="TT")
                Gv = G[:].rearrange("p f x k e -> p f (x k) e")
                W8b = W8[:].rearrange("p f k t -> p f (k t)").unsqueeze(-1).broadcast_to([P, F, 8, FEAT])
                nc.vector.tensor_tensor(out=TT[:], in0=Gv, in1=W8b, op=Op.mult)
                Tf = TT[:].rearrange("p f d e -> p (f d e)")
                for width in (72, 36, 18):
                    a = Tf.rearrange("p (f e) -> p f e", e=144)[:, :, 0:width]
                    bb = Tf.rearrange("p (f e) -> p f e", e=144)[:, :, width : 2 * width]
                    nc.vector.tensor_tensor(out=a, in0=a, in1=bb, op=Op.add)

                OUTT = spool.tile([P, F, FEAT], f32, tag="OUTT")
                nc.vector.tensor_copy(
                    out=OUTT[:],
                    in_=Tf.rearrange("p (f e) -> p f e", e=144)[:, :, 0:FEAT])
                nc.sync.dma_start(
                    out=out[c0 : c0 + chunk, :].rearrange("(p f) c -> p (f c)", p=P),
                    in_=OUTT[:].rearrange("p f e -> p (f e)"),
                )
    nc.finalize()
    return nc


def kernel(pts, codebook, transform, _trace=False):
    from concourse.bass_utils import run_bass_kernel_spmd

    pts = np.asarray(pts, dtype=np.float32)
    codebook = np.ascontiguousarray(np.asarray(codebook, dtype=np.float32))
    transform = np.asarray(transform, dtype=np.float32)

    p_flat = np.ascontiguousarray(pts.reshape(-1, 3))
    n_total = p_flat.shape[0]
    n_per = n_total // N_CORES
    assert n_per * N_CORES == n_total

    # fold transform inverse + grid scale into affine q = A p + b (host side,
    # 4x4 input only)
    R_inv = np.linalg.inv(transform[:3, :3].astype(np.float64))
    A = (RES - 1) * R_inv
    b = -A @ transform[:3, 3].astype(np.float64)

    key = (n_per, A.tobytes(), b.tobytes())
    if key not in _cache:
        _cache[key] = _build(n_per, A, b)
    nc = _cache[key]

    in_maps = [
        {"pts": p_flat[i * n_per : (i + 1) * n_per], "codebook": codebook}
        for i in range(N_CORES)
    ]
    r = run_bass_kernel_spmd(nc, in_maps, list(range(N_CORES)), trace=_trace)
    kernel.last_exec_time_ns = r.exec_time_ns
    out = np.concatenate([r.results[i]["out"] for i in range(N_CORES)], axis=0)
    return out


kernel.last_exec_time_ns = None
